# revision 1
# baseline (speedup 1.0000x reference)
"""ChildSum TreeLSTM on a complete binary tree (131071 nodes, depth 17),
distributed over 8 trn2 NeuronCores.

Sharding: core k owns the subtree rooted at level-3 node (7+k): levels 16..3
split contiguously 8 ways -> zero cross-core traffic. Host computes levels
2..0 (7 nodes) in numpy.

Device layout: everything feature-major [feat(part), node(free)].  Each
level's nodes are stored in "children-split" order (s_3=[0];
s_{l+1}=[2i for i in s_l]+[2i+1 ...]) so a parent at stored pos j has its
left child at child-stored pos j and right child at pos n_parent+j ->
all child access is two contiguous slices.
"""
import sys
import numpy as np

for _p in ('/opt/trn_rl_repo',):
    if _p not in sys.path:
        sys.path.insert(0, _p)

N_NODES, D, P = 131071, 256, 128
NCORES = 8
LVLS = list(range(16, 2, -1))            # per-core levels 16..3
CNT = {l: (2 ** l) // NCORES for l in LVLS}
SEG = {}
_off = 0
for _l in LVLS:
    SEG[_l] = _off
    _off += CNT[_l]
NLOC = _off            # 16383
NLOC_PAD = 16384
NT = 512               # node tile width


def stored_orders():
    s = {3: np.array([0], dtype=np.int64)}
    for l in range(3, 17):
        s[l + 1] = np.concatenate([2 * s[l], 2 * s[l] + 1])
    return s


_PROGRAM_CACHE = {}


def build_program(repeat=1, pool_off=False):
    key = ('nc', repeat, pool_off)
    if key in _PROGRAM_CACHE:
        return _PROGRAM_CACHE[key]
    import concourse.bacc as bacc
    import concourse.mybir as mybir
    import concourse.tile as tile
    from contextlib import ExitStack, nullcontext

    f32 = mybir.dt.float32
    f32r = mybir.dt.float32r
    AF = mybir.ActivationFunctionType

    nc = bacc.Bacc("TRN2", target_bir_lowering=False, debug=False,
                   num_devices=NCORES)

    x_d = nc.dram_tensor("x", [2, P, NLOC_PAD], f32r, kind="ExternalInput").ap()
    wx_d = nc.dram_tensor("wioux", [2, P, 768], f32r, kind="ExternalInput").ap()
    wh_d = nc.dram_tensor("wiouh", [2, P, 768], f32r, kind="ExternalInput").ap()
    wfx_d = nc.dram_tensor("wfx", [2, P, 256], f32r, kind="ExternalInput").ap()
    wfh_d = nc.dram_tensor("wfh", [2, P, 256], f32r, kind="ExternalInput").ap()
    bio_d = nc.dram_tensor("bio", [P, 6], f32, kind="ExternalInput").ap()
    bf_d = nc.dram_tensor("bf", [P, 2], f32, kind="ExternalInput").ap()
    out_d = nc.dram_tensor("out", [2, 2, P, 1], f32, kind="ExternalOutput").ap()

    with tile.TileContext(nc) as tc, ExitStack() as ctx:
        wpool = ctx.enter_context(tc.tile_pool(name="w", bufs=1))
        hcpool = ctx.enter_context(tc.tile_pool(name="hc", bufs=1))
        xpool = ctx.enter_context(tc.tile_pool(name="xp", bufs=6))
        gpool = ctx.enter_context(tc.tile_pool(name="gp", bufs=1))
        ppool = ctx.enter_context(tc.tile_pool(name="pp", bufs=6, space="PSUM"))
        fpool = ctx.enter_context(tc.tile_pool(name="pf", bufs=1, space="PSUM"))

        # ---- weights / biases in SBUF (persistent) ----
        WX, WH, WFX, WFH = [], [], [], []
        for c in range(2):
            t = wpool.tile([P, 768], f32r, name=f"wxs{c}")
            nc.sync.dma_start(t[:], wx_d[c])
            WX.append(t)
            t = wpool.tile([P, 768], f32r, name=f"whs{c}")
            nc.sync.dma_start(t[:], wh_d[c])
            WH.append(t)
            t = wpool.tile([P, 256], f32r, name=f"wfxs{c}")
            nc.sync.dma_start(t[:], wfx_d[c])
            WFX.append(t)
            t = wpool.tile([P, 256], f32r, name=f"wfhs{c}")
            nc.sync.dma_start(t[:], wfh_d[c])
            WFH.append(t)
        BIO = wpool.tile([P, 6], f32, name="bios")
        nc.sync.dma_start(BIO[:], bio_d[:])
        BF = wpool.tile([P, 2], f32, name="bfs")
        nc.sync.dma_start(BF[:], bf_d[:])

        # ---- persistent H/C buffers: lvl14 + ping-pong for 13..3 ----
        H14 = [hcpool.tile([P, 2048], f32r, name=f"H14_{c}") for c in range(2)]
        C14 = [hcpool.tile([P, 2048], f32, name=f"C14_{c}") for c in range(2)]
        HA = [hcpool.tile([P, 1024], f32r, name=f"HA{c}") for c in range(2)]
        CA = [hcpool.tile([P, 1024], f32, name=f"CA{c}") for c in range(2)]
        HB = [hcpool.tile([P, 512], f32r, name=f"HB{c}") for c in range(2)]
        CB = [hcpool.tile([P, 512], f32, name=f"CB{c}") for c in range(2)]

        def bufs_for(lvl):
            if lvl == 14:
                return (H14, C14)
            return (HA, CA) if lvl % 2 == 1 else (HB, CB)

        loop_cm = tc.For_i(0, repeat, 1) if repeat > 1 else nullcontext()

        def load_x(lvl, j, m):
            xs = []
            for c in range(2):
                xt = xpool.tile([P, NT], f32r, name="xt", tag="xt")
                nc.sync.dma_start(xt[:, :m],
                                  x_d[c, :, SEG[lvl] + j: SEG[lvl] + j + m])
                xs.append(xt)
            return xs

        def unit_leaf(xs, m, houts, couts):
            """leaf recurrence for m nodes; writes into houts/couts AP slices"""
            gates = []
            for fo in range(6):
                pt = ppool.tile([P, NT], f32, name="pt", tag="pt")
                nc.tensor.matmul(pt[:, :m], WX[0][:, fo * P:(fo + 1) * P],
                                 xs[0][:, :m], start=True, stop=False)
                nc.tensor.matmul(pt[:, :m], WX[1][:, fo * P:(fo + 1) * P],
                                 xs[1][:, :m], start=False, stop=True)
                g = gpool.tile([P, NT], f32, name="lg", tag=f"lg{fo}", bufs=2)
                func = AF.Tanh if fo >= 4 else AF.Sigmoid
                nc.scalar.activation(g[:, :m], pt[:, :m], func,
                                     bias=BIO[:, fo:fo + 1])
                gates.append(g)
            for c in range(2):
                ct = couts(c)
                nc.vector.tensor_mul(ct, gates[c][:, :m], gates[4 + c][:, :m])
                tt = gpool.tile([P, NT], f32, name="lt", tag=f"tmp{c}")
                nc.scalar.activation(tt[:, :m], ct, AF.Tanh)
                nc.vector.tensor_mul(houts(c), gates[2 + c][:, :m], tt[:, :m])

        def unit_internal(xs, hL, cL, hR, cR, houts, couts, m):
            """internal recurrence for m nodes; children/outputs via callables"""
            hsum = []
            for c in range(2):
                hs = gpool.tile([P, NT], f32r, name="hs", tag=f"hs{c}")
                if pool_off:
                    nc.gpsimd.tensor_add(hs[:, :m], hL(c), hR(c))
                else:
                    nc.vector.tensor_add(hs[:, :m], hL(c), hR(c))
                hsum.append(hs)
            gates = []
            for fo in range(6):
                pt = ppool.tile([P, NT], f32, name="pt", tag="pt")
                nc.tensor.matmul(pt[:, :m], WX[0][:, fo * P:(fo + 1) * P],
                                 xs[0][:, :m], start=True, stop=False)
                nc.tensor.matmul(pt[:, :m], WX[1][:, fo * P:(fo + 1) * P],
                                 xs[1][:, :m], start=False, stop=False)
                nc.tensor.matmul(pt[:, :m], WH[0][:, fo * P:(fo + 1) * P],
                                 hsum[0][:, :m], start=False, stop=False)
                nc.tensor.matmul(pt[:, :m], WH[1][:, fo * P:(fo + 1) * P],
                                 hsum[1][:, :m], start=False, stop=True)
                g = gpool.tile([P, NT], f32, name="ig", tag=f"ig{fo}", bufs=2)
                func = AF.Tanh if fo >= 4 else AF.Sigmoid
                nc.scalar.activation(g[:, :m], pt[:, :m], func,
                                     bias=BIO[:, fo:fo + 1])
                gates.append(g)
            # forget gates: [fL | fR] share one 2-bank psum tile per fo chunk
            # (same per-partition bias and same sigmoid for both halves)
            fg = []
            for fo in range(2):
                pf = fpool.tile([P, 2 * NT], f32, name="pf", tag="pf")
                for half, hh in ((0, hL), (1, hR)):
                    sl = slice(half * m, half * m + m)
                    nc.tensor.matmul(pf[:, sl],
                                     WFH[0][:, fo * P:(fo + 1) * P],
                                     hh(0), start=True, stop=False)
                    nc.tensor.matmul(pf[:, sl],
                                     WFH[1][:, fo * P:(fo + 1) * P],
                                     hh(1), start=False, stop=False)
                    nc.tensor.matmul(pf[:, sl],
                                     WFX[0][:, fo * P:(fo + 1) * P],
                                     xs[0][:, :m], start=False, stop=False)
                    nc.tensor.matmul(pf[:, sl],
                                     WFX[1][:, fo * P:(fo + 1) * P],
                                     xs[1][:, :m], start=False, stop=True)
                g = gpool.tile([P, 2 * NT], f32, name="fgt", tag=f"fgt{fo}")
                nc.scalar.activation(g[:, :2 * m], pf[:, :2 * m], AF.Sigmoid,
                                     bias=BF[:, fo:fo + 1])
                fg.append(g)
            for c in range(2):
                t1 = gpool.tile([P, NT], f32, name="t1", tag=f"t1{c}")
                nc.vector.tensor_mul(t1[:, :m], fg[c][:, :m], cL(c))
                t2 = gpool.tile([P, NT], f32, name="t2", tag=f"t2{c}")
                if pool_off:
                    nc.gpsimd.tensor_mul(t2[:, :m], fg[c][:, m:2 * m], cR(c))
                else:
                    nc.vector.tensor_mul(t2[:, :m], fg[c][:, m:2 * m], cR(c))
                nc.vector.tensor_add(t1[:, :m], t1[:, :m], t2[:, :m])
                t3 = gpool.tile([P, NT], f32, name="t3", tag=f"t3{c}")
                nc.vector.tensor_mul(t3[:, :m], gates[c][:, :m],
                                     gates[4 + c][:, :m])
                cn = couts(c)
                nc.vector.tensor_add(cn, t3[:, :m], t1[:, :m])
                tt = gpool.tile([P, NT], f32, name="tt", tag=f"tt{c}")
                nc.scalar.activation(tt[:, :m], cn, AF.Tanh)
                nc.vector.tensor_mul(houts(c), gates[2 + c][:, :m], tt[:, :m])

        def sl_w(bufs, c, j, m):
            return bufs[c][:, j:j + m]

        with loop_cm:
            # ---- phase 1: leaves + lvl15 fused into lvl14 tiles ----
            for t in range(CNT[14] // NT):
                p15 = {}
                for u, s in (("a", t * NT), ("b", CNT[15] // 2 + t * NT)):
                    lhv, lcv = {}, {}
                    for S, js in (("L", s), ("R", CNT[16] // 2 + s)):
                        xsL = load_x(16, js, NT)
                        hts = [gpool.tile([P, NT], f32r, name="e16h",
                                          tag=f"e16h{S}{c}") for c in range(2)]
                        cts = [gpool.tile([P, NT], f32, name="e16c",
                                          tag=f"e16c{S}{c}") for c in range(2)]
                        unit_leaf(xsL, NT, lambda c: hts[c][:, :NT],
                                  lambda c: cts[c][:, :NT])
                        lhv[S], lcv[S] = hts, cts
                    xs15 = load_x(15, s, NT)
                    h15 = [gpool.tile([P, NT], f32r, name="p15h",
                                      tag=f"p15h{u}{c}") for c in range(2)]
                    c15 = [gpool.tile([P, NT], f32, name="p15c",
                                      tag=f"p15c{u}{c}") for c in range(2)]
                    unit_internal(xs15,
                                  lambda c: lhv["L"][c][:, :NT],
                                  lambda c: lcv["L"][c][:, :NT],
                                  lambda c: lhv["R"][c][:, :NT],
                                  lambda c: lcv["R"][c][:, :NT],
                                  lambda c: h15[c][:, :NT],
                                  lambda c: c15[c][:, :NT], NT)
                    p15[u] = (h15, c15)
                xs14 = load_x(14, t * NT, NT)
                j = t * NT
                unit_internal(xs14,
                              lambda c: p15["a"][0][c][:, :NT],
                              lambda c: p15["a"][1][c][:, :NT],
                              lambda c: p15["b"][0][c][:, :NT],
                              lambda c: p15["b"][1][c][:, :NT],
                              lambda c: H14[c][:, j:j + NT],
                              lambda c: C14[c][:, j:j + NT], NT)

            # ---- phase 2: levels 13..3 ----
            for lvl in range(13, 2, -1):
                n = CNT[lvl]
                HC, CC = bufs_for(lvl + 1)
                HO, CO = bufs_for(lvl)
                for j in range(0, n, NT):
                    # fp32r matmul needs even sizes; pad the 1-node level-3
                    # tile to 2 (junk columns never read: x is padded, H/C
                    # buffers oversized)
                    m = max(min(NT, n - j), 2)
                    xsP = load_x(lvl, j, m)
                    unit_internal(
                        xsP,
                        lambda c: HC[c][:, j:j + m],
                        lambda c: CC[c][:, j:j + m],
                        lambda c: HC[c][:, n + j:n + j + m],
                        lambda c: CC[c][:, n + j:n + j + m],
                        lambda c: HO[c][:, j:j + m],
                        lambda c: CO[c][:, j:j + m], m)

            # ---- output: level-3 root of this core's subtree ----
            H3, C3 = bufs_for(3)
            for c in range(2):
                nc.sync.dma_start(out_d[0, c], H3[c][:, 0:1].bitcast(f32))
                nc.sync.dma_start(out_d[1, c], C3[c][:, 0:1])

    nc.compile()
    _PROGRAM_CACHE[key] = nc
    return nc


def shard_inputs(inputs, W_ioux, b_ioux, W_iouh, b_iouh, W_fx, b_fx, W_fh, b_fh):
    """Build per-core input maps."""
    so = stored_orders()
    f32 = np.float32
    wioux = np.ascontiguousarray(np.asarray(W_ioux, f32).T.reshape(2, P, 768))
    wiouh = np.ascontiguousarray(np.asarray(W_iouh, f32).T.reshape(2, P, 768))
    wfx = np.ascontiguousarray(np.asarray(W_fx, f32).T.reshape(2, P, 256))
    wfh = np.ascontiguousarray(np.asarray(W_fh, f32).T.reshape(2, P, 256))
    bio = np.ascontiguousarray((np.asarray(b_ioux, f32)
                                + np.asarray(b_iouh, f32)).reshape(6, P).T)
    bf = np.ascontiguousarray((np.asarray(b_fx, f32)
                               + np.asarray(b_fh, f32)).reshape(2, P).T)
    inputs = np.asarray(inputs, f32)

    in_maps = []
    for k in range(NCORES):
        xk = np.zeros((NLOC_PAD, D), dtype=f32)
        for l in LVLS:
            n = CNT[l]
            gs = 2 ** l - 1 + k * n
            xk[SEG[l]:SEG[l] + n] = inputs[gs:gs + n][so[l]]
        xk = np.ascontiguousarray(xk.T).reshape(2, P, NLOC_PAD)
        in_maps.append({
            "x": xk, "wioux": wioux, "wiouh": wiouh, "wfx": wfx, "wfh": wfh,
            "bio": bio, "bf": bf,
        })
    return in_maps


def _sig(v):
    return 1.0 / (1.0 + np.exp(-v))


def core_reference(k, inputs, W_ioux, b_ioux, W_iouh, b_iouh,
                   W_fx, b_fx, W_fh, b_fh):
    """numpy emulation of what core k should output (h3, c3), fp64-ish."""
    f32 = np.float32
    so = stored_orders()
    h = c = None
    for l in range(16, 2, -1):
        n = CNT[l]
        gs = 2 ** l - 1 + k * n
        x = np.asarray(inputs[gs:gs + n], f32)[so[l]]
        iou = x @ W_ioux.T + b_ioux + b_iouh
        if l < 16:
            hsum = h[:n] + h[n:]
            iou = iou + hsum @ W_iouh.T
            fL = _sig(h[:n] @ W_fh.T + x @ W_fx.T + b_fx + b_fh)
            fR = _sig(h[n:] @ W_fh.T + x @ W_fx.T + b_fx + b_fh)
            fc = fL * c[:n] + fR * c[n:]
        else:
            fc = 0.0
        i, o, u = np.split(iou, 3, axis=1)
        cn = _sig(i) * np.tanh(u) + fc
        hn = _sig(o) * np.tanh(cn)
        h, c = hn, cn
    return h[0], c[0]


def top_of_tree(h3, c3, inputs, W_ioux, b_ioux, W_iouh, b_iouh,
                W_fx, b_fx, W_fh, b_fh):
    """numpy levels 2..0. h3/c3: [8, 256] states of nodes 7..14."""
    f32 = np.float32
    h = np.zeros((15, D), dtype=f32)
    c = np.zeros((15, D), dtype=f32)
    h[7:15] = h3
    c[7:15] = c3
    x = np.asarray(inputs[:7], f32)
    iou_x = x @ np.asarray(W_ioux, f32).T + b_ioux
    fx = x @ np.asarray(W_fx, f32).T + b_fx

    for lvl in (2, 1, 0):
        start, count = 2 ** lvl - 1, 2 ** lvl
        cs = 2 * start + 1
        ch = h[cs:cs + 2 * count].reshape(count, 2, D)
        cc = c[cs:cs + 2 * count].reshape(count, 2, D)
        iou = iou_x[start:start + count] + ch.sum(axis=1) @ W_iouh.T + b_iouh
        f = _sig(np.einsum("nkm,pm->nkp", ch, W_fh) + b_fh
                 + fx[start:start + count][:, None, :])
        fc_sum = (f * cc).sum(axis=1)
        i, o, u = np.split(iou, 3, axis=1)
        c_new = _sig(i) * np.tanh(u) + fc_sum
        h_new = _sig(o) * np.tanh(c_new)
        c[start:start + count] = c_new
        h[start:start + count] = h_new
    return c[0:1].astype(f32), h[0:1].astype(f32)


def run_device(in_maps, trace=False, repeat=1, pool_off=False):
    from concourse.bass_utils import run_bass_kernel_spmd
    nc = build_program(repeat, pool_off)
    return run_bass_kernel_spmd(nc, in_maps, core_ids=list(range(NCORES)),
                                trace=trace)


def kernel(inputs, W_ioux, b_ioux, W_iouh, b_iouh, W_fx, b_fx, W_fh, b_fh):
    args = (inputs, W_ioux, b_ioux, W_iouh, b_iouh, W_fx, b_fx, W_fh, b_fh)
    in_maps = shard_inputs(*args)
    res = run_device(in_maps)
    h3 = np.stack([res.results[k]["out"][0].reshape(D) for k in range(NCORES)])
    c3 = np.stack([res.results[k]["out"][1].reshape(D) for k in range(NCORES)])
    return top_of_tree(h3, c3, *args)



# revision 4
# speedup vs baseline: 2.2620x; 2.2620x over previous
"""ChildSum TreeLSTM on a complete binary tree (131071 nodes, depth 17),
distributed over 8 trn2 NeuronCores.

Sharding: core k owns the subtree rooted at level-3 node (7+k): levels 16..3
split contiguously 8 ways -> zero cross-core traffic. Host computes levels
2..0 (7 nodes) in numpy.

Device layout: everything feature-major [feat(part), node(free)].  Each
level's nodes are stored in "children-split" order (s_3=[0];
s_{l+1}=[2i for i in s_l]+[2i+1 ...]) so a parent at stored pos j has its
left child at child-stored pos j and right child at pos n_parent+j ->
all child access is two contiguous slices.
"""
import sys
import numpy as np

for _p in ('/opt/trn_rl_repo',):
    if _p not in sys.path:
        sys.path.insert(0, _p)

N_NODES, D, P = 131071, 256, 128
NCORES = 8
LVLS = list(range(16, 2, -1))            # per-core levels 16..3
CNT = {l: (2 ** l) // NCORES for l in LVLS}
SEG = {}
_off = 0
for _l in LVLS:
    SEG[_l] = _off
    _off += CNT[_l]
NLOC = _off            # 16383
NLOC_PAD = 16384
NT = 512               # node tile width


def stored_orders():
    s = {3: np.array([0], dtype=np.int64)}
    for l in range(3, 17):
        s[l + 1] = np.concatenate([2 * s[l], 2 * s[l] + 1])
    return s


_PROGRAM_CACHE = {}


def build_program(repeat=1, pool_off=False):
    key = ('nc', repeat, pool_off)
    if key in _PROGRAM_CACHE:
        return _PROGRAM_CACHE[key]
    import concourse.bacc as bacc
    import concourse.mybir as mybir
    import concourse.tile as tile
    from contextlib import ExitStack, nullcontext

    f32 = mybir.dt.float32
    bf16 = mybir.dt.bfloat16
    AF = mybir.ActivationFunctionType

    nc = bacc.Bacc("TRN2", target_bir_lowering=False, debug=False,
                   num_devices=NCORES)

    x_d = nc.dram_tensor("x", [2, P, NLOC_PAD], bf16, kind="ExternalInput").ap()
    wx_d = nc.dram_tensor("wioux", [2, P, 768], bf16, kind="ExternalInput").ap()
    wh_d = nc.dram_tensor("wiouh", [2, P, 768], bf16, kind="ExternalInput").ap()
    wfx_d = nc.dram_tensor("wfx", [2, P, 256], bf16, kind="ExternalInput").ap()
    wfh_d = nc.dram_tensor("wfh", [2, P, 256], bf16, kind="ExternalInput").ap()
    bio_d = nc.dram_tensor("bio", [P, 6], f32, kind="ExternalInput").ap()
    bf_d = nc.dram_tensor("bf", [P, 2], f32, kind="ExternalInput").ap()
    out_d = nc.dram_tensor("out", [2, 2, P, 1], f32, kind="ExternalOutput").ap()

    with tile.TileContext(nc) as tc, ExitStack() as ctx:
        wpool = ctx.enter_context(tc.tile_pool(name="w", bufs=1))
        hcpool = ctx.enter_context(tc.tile_pool(name="hc", bufs=1))
        xpool = ctx.enter_context(tc.tile_pool(name="xp", bufs=6))
        gpool = ctx.enter_context(tc.tile_pool(name="gp", bufs=1))
        ppool = ctx.enter_context(tc.tile_pool(name="pp", bufs=6, space="PSUM"))
        fpool = ctx.enter_context(tc.tile_pool(name="pf", bufs=1, space="PSUM"))

        # ---- weights / biases in SBUF (persistent) ----
        WX, WH, WFX, WFH = [], [], [], []
        for c in range(2):
            t = wpool.tile([P, 768], bf16, name=f"wxs{c}")
            nc.sync.dma_start(t[:], wx_d[c])
            WX.append(t)
            t = wpool.tile([P, 768], bf16, name=f"whs{c}")
            nc.sync.dma_start(t[:], wh_d[c])
            WH.append(t)
            t = wpool.tile([P, 256], bf16, name=f"wfxs{c}")
            nc.sync.dma_start(t[:], wfx_d[c])
            WFX.append(t)
            t = wpool.tile([P, 256], bf16, name=f"wfhs{c}")
            nc.sync.dma_start(t[:], wfh_d[c])
            WFH.append(t)
        BIO = wpool.tile([P, 6], f32, name="bios")
        nc.sync.dma_start(BIO[:], bio_d[:])
        BF = wpool.tile([P, 2], f32, name="bfs")
        nc.sync.dma_start(BF[:], bf_d[:])

        # ---- persistent H/C buffers: lvl14 + ping-pong for 13..3 ----
        H14 = [hcpool.tile([P, 2048], bf16, name=f"H14_{c}") for c in range(2)]
        C14 = [hcpool.tile([P, 2048], f32, name=f"C14_{c}") for c in range(2)]
        HA = [hcpool.tile([P, 1024], bf16, name=f"HA{c}") for c in range(2)]
        CA = [hcpool.tile([P, 1024], f32, name=f"CA{c}") for c in range(2)]
        HB = [hcpool.tile([P, 512], bf16, name=f"HB{c}") for c in range(2)]
        CB = [hcpool.tile([P, 512], f32, name=f"CB{c}") for c in range(2)]

        def bufs_for(lvl):
            if lvl == 14:
                return (H14, C14)
            return (HA, CA) if lvl % 2 == 1 else (HB, CB)

        loop_cm = tc.For_i(0, repeat, 1) if repeat > 1 else nullcontext()

        def load_x(lvl, j, m):
            xs = []
            for c in range(2):
                xt = xpool.tile([P, NT], bf16, name="xt", tag="xt")
                nc.sync.dma_start(xt[:, :m],
                                  x_d[c, :, SEG[lvl] + j: SEG[lvl] + j + m])
                xs.append(xt)
            return xs

        def unit_leaf(xs, m, houts, couts):
            """leaf recurrence for m nodes; writes into houts/couts AP slices"""
            gates = []
            for fo in range(6):
                pt = ppool.tile([P, NT], f32, name="pt", tag="pt")
                nc.tensor.matmul(pt[:, :m], WX[0][:, fo * P:(fo + 1) * P],
                                 xs[0][:, :m], start=True, stop=False)
                nc.tensor.matmul(pt[:, :m], WX[1][:, fo * P:(fo + 1) * P],
                                 xs[1][:, :m], start=False, stop=True)
                g = gpool.tile([P, NT], f32, name="lg", tag=f"lg{fo}", bufs=2)
                func = AF.Tanh if fo >= 4 else AF.Sigmoid
                nc.scalar.activation(g[:, :m], pt[:, :m], func,
                                     bias=BIO[:, fo:fo + 1])
                gates.append(g)
            for c in range(2):
                ct = couts(c)
                nc.vector.tensor_mul(ct, gates[c][:, :m], gates[4 + c][:, :m])
                tt = gpool.tile([P, NT], f32, name="lt", tag=f"tmp{c}")
                nc.scalar.activation(tt[:, :m], ct, AF.Tanh)
                nc.vector.tensor_mul(houts(c), gates[2 + c][:, :m], tt[:, :m])

        def unit_internal(xs, hL, cL, hR, cR, houts, couts, m):
            """internal recurrence for m nodes; children/outputs via callables"""
            hsum = []
            for c in range(2):
                hs = gpool.tile([P, NT], bf16, name="hs", tag=f"hs{c}")
                if pool_off:
                    nc.gpsimd.tensor_add(hs[:, :m], hL(c), hR(c))
                else:
                    nc.vector.tensor_add(hs[:, :m], hL(c), hR(c))
                hsum.append(hs)
            gates = []
            for fo in range(6):
                pt = ppool.tile([P, NT], f32, name="pt", tag="pt")
                nc.tensor.matmul(pt[:, :m], WX[0][:, fo * P:(fo + 1) * P],
                                 xs[0][:, :m], start=True, stop=False)
                nc.tensor.matmul(pt[:, :m], WX[1][:, fo * P:(fo + 1) * P],
                                 xs[1][:, :m], start=False, stop=False)
                nc.tensor.matmul(pt[:, :m], WH[0][:, fo * P:(fo + 1) * P],
                                 hsum[0][:, :m], start=False, stop=False)
                nc.tensor.matmul(pt[:, :m], WH[1][:, fo * P:(fo + 1) * P],
                                 hsum[1][:, :m], start=False, stop=True)
                g = gpool.tile([P, NT], f32, name="ig", tag=f"ig{fo}", bufs=2)
                func = AF.Tanh if fo >= 4 else AF.Sigmoid
                nc.scalar.activation(g[:, :m], pt[:, :m], func,
                                     bias=BIO[:, fo:fo + 1])
                gates.append(g)
            # forget gates: [fL | fR] share one 2-bank psum tile per fo chunk
            # (same per-partition bias and same sigmoid for both halves)
            fg = []
            for fo in range(2):
                pf = fpool.tile([P, 2 * NT], f32, name="pf", tag="pf")
                for half, hh in ((0, hL), (1, hR)):
                    sl = slice(half * m, half * m + m)
                    nc.tensor.matmul(pf[:, sl],
                                     WFH[0][:, fo * P:(fo + 1) * P],
                                     hh(0), start=True, stop=False)
                    nc.tensor.matmul(pf[:, sl],
                                     WFH[1][:, fo * P:(fo + 1) * P],
                                     hh(1), start=False, stop=False)
                    nc.tensor.matmul(pf[:, sl],
                                     WFX[0][:, fo * P:(fo + 1) * P],
                                     xs[0][:, :m], start=False, stop=False)
                    nc.tensor.matmul(pf[:, sl],
                                     WFX[1][:, fo * P:(fo + 1) * P],
                                     xs[1][:, :m], start=False, stop=True)
                g = gpool.tile([P, 2 * NT], f32, name="fgt", tag=f"fgt{fo}")
                nc.scalar.activation(g[:, :2 * m], pf[:, :2 * m], AF.Sigmoid,
                                     bias=BF[:, fo:fo + 1])
                fg.append(g)
            for c in range(2):
                t1 = gpool.tile([P, NT], f32, name="t1", tag=f"t1{c}")
                nc.vector.tensor_mul(t1[:, :m], fg[c][:, :m], cL(c))
                t2 = gpool.tile([P, NT], f32, name="t2", tag=f"t2{c}")
                if pool_off:
                    nc.gpsimd.tensor_mul(t2[:, :m], fg[c][:, m:2 * m], cR(c))
                else:
                    nc.vector.tensor_mul(t2[:, :m], fg[c][:, m:2 * m], cR(c))
                nc.vector.tensor_add(t1[:, :m], t1[:, :m], t2[:, :m])
                t3 = gpool.tile([P, NT], f32, name="t3", tag=f"t3{c}")
                nc.vector.tensor_mul(t3[:, :m], gates[c][:, :m],
                                     gates[4 + c][:, :m])
                cn = couts(c)
                nc.vector.tensor_add(cn, t3[:, :m], t1[:, :m])
                tt = gpool.tile([P, NT], f32, name="tt", tag=f"tt{c}")
                nc.scalar.activation(tt[:, :m], cn, AF.Tanh)
                nc.vector.tensor_mul(houts(c), gates[2 + c][:, :m], tt[:, :m])

        def sl_w(bufs, c, j, m):
            return bufs[c][:, j:j + m]

        with loop_cm:
            # ---- phase 1: leaves + lvl15 fused into lvl14 tiles ----
            for t in range(CNT[14] // NT):
                p15 = {}
                for u, s in (("a", t * NT), ("b", CNT[15] // 2 + t * NT)):
                    lhv, lcv = {}, {}
                    for S, js in (("L", s), ("R", CNT[16] // 2 + s)):
                        xsL = load_x(16, js, NT)
                        hts = [gpool.tile([P, NT], bf16, name="e16h",
                                          tag=f"e16h{S}{c}") for c in range(2)]
                        cts = [gpool.tile([P, NT], f32, name="e16c",
                                          tag=f"e16c{S}{c}") for c in range(2)]
                        unit_leaf(xsL, NT, lambda c: hts[c][:, :NT],
                                  lambda c: cts[c][:, :NT])
                        lhv[S], lcv[S] = hts, cts
                    xs15 = load_x(15, s, NT)
                    h15 = [gpool.tile([P, NT], bf16, name="p15h",
                                      tag=f"p15h{u}{c}") for c in range(2)]
                    c15 = [gpool.tile([P, NT], f32, name="p15c",
                                      tag=f"p15c{u}{c}") for c in range(2)]
                    unit_internal(xs15,
                                  lambda c: lhv["L"][c][:, :NT],
                                  lambda c: lcv["L"][c][:, :NT],
                                  lambda c: lhv["R"][c][:, :NT],
                                  lambda c: lcv["R"][c][:, :NT],
                                  lambda c: h15[c][:, :NT],
                                  lambda c: c15[c][:, :NT], NT)
                    p15[u] = (h15, c15)
                xs14 = load_x(14, t * NT, NT)
                j = t * NT
                unit_internal(xs14,
                              lambda c: p15["a"][0][c][:, :NT],
                              lambda c: p15["a"][1][c][:, :NT],
                              lambda c: p15["b"][0][c][:, :NT],
                              lambda c: p15["b"][1][c][:, :NT],
                              lambda c: H14[c][:, j:j + NT],
                              lambda c: C14[c][:, j:j + NT], NT)

            # ---- phase 2: levels 13..3 ----
            for lvl in range(13, 2, -1):
                n = CNT[lvl]
                HC, CC = bufs_for(lvl + 1)
                HO, CO = bufs_for(lvl)
                for j in range(0, n, NT):
                    # fp32r matmul needs even sizes; pad the 1-node level-3
                    # tile to 2 (junk columns never read: x is padded, H/C
                    # buffers oversized)
                    m = max(min(NT, n - j), 2)
                    xsP = load_x(lvl, j, m)
                    unit_internal(
                        xsP,
                        lambda c: HC[c][:, j:j + m],
                        lambda c: CC[c][:, j:j + m],
                        lambda c: HC[c][:, n + j:n + j + m],
                        lambda c: CC[c][:, n + j:n + j + m],
                        lambda c: HO[c][:, j:j + m],
                        lambda c: CO[c][:, j:j + m], m)

            # ---- output: level-3 root of this core's subtree ----
            H3, C3 = bufs_for(3)
            HOUT = wpool.tile([P, 2], f32, name="hout")
            for c in range(2):
                nc.scalar.copy(HOUT[:, c:c + 1], H3[c][:, 0:1])
                nc.sync.dma_start(out_d[0, c], HOUT[:, c:c + 1])
                nc.sync.dma_start(out_d[1, c], C3[c][:, 0:1])

    nc.compile()
    _PROGRAM_CACHE[key] = nc
    return nc


def shard_inputs(inputs, W_ioux, b_ioux, W_iouh, b_iouh, W_fx, b_fx, W_fh, b_fh):
    """Build per-core input maps."""
    from ml_dtypes import bfloat16
    so = stored_orders()
    f32 = np.float32
    wioux = np.ascontiguousarray(
        np.asarray(W_ioux, f32).T.reshape(2, P, 768)).astype(bfloat16)
    wiouh = np.ascontiguousarray(
        np.asarray(W_iouh, f32).T.reshape(2, P, 768)).astype(bfloat16)
    wfx = np.ascontiguousarray(
        np.asarray(W_fx, f32).T.reshape(2, P, 256)).astype(bfloat16)
    wfh = np.ascontiguousarray(
        np.asarray(W_fh, f32).T.reshape(2, P, 256)).astype(bfloat16)
    bio = np.ascontiguousarray((np.asarray(b_ioux, f32)
                                + np.asarray(b_iouh, f32)).reshape(6, P).T)
    bf = np.ascontiguousarray((np.asarray(b_fx, f32)
                               + np.asarray(b_fh, f32)).reshape(2, P).T)
    inputs = np.asarray(inputs, f32)

    in_maps = []
    for k in range(NCORES):
        xk = np.zeros((NLOC_PAD, D), dtype=f32)
        for l in LVLS:
            n = CNT[l]
            gs = 2 ** l - 1 + k * n
            xk[SEG[l]:SEG[l] + n] = inputs[gs:gs + n][so[l]]
        xk = np.ascontiguousarray(xk.T).reshape(2, P, NLOC_PAD).astype(bfloat16)
        in_maps.append({
            "x": xk, "wioux": wioux, "wiouh": wiouh, "wfx": wfx, "wfh": wfh,
            "bio": bio, "bf": bf,
        })
    return in_maps


def _sig(v):
    return 1.0 / (1.0 + np.exp(-v))


def core_reference(k, inputs, W_ioux, b_ioux, W_iouh, b_iouh,
                   W_fx, b_fx, W_fh, b_fh):
    """numpy emulation of what core k should output (h3, c3), fp64-ish."""
    f32 = np.float32
    so = stored_orders()
    h = c = None
    for l in range(16, 2, -1):
        n = CNT[l]
        gs = 2 ** l - 1 + k * n
        x = np.asarray(inputs[gs:gs + n], f32)[so[l]]
        iou = x @ W_ioux.T + b_ioux + b_iouh
        if l < 16:
            hsum = h[:n] + h[n:]
            iou = iou + hsum @ W_iouh.T
            fL = _sig(h[:n] @ W_fh.T + x @ W_fx.T + b_fx + b_fh)
            fR = _sig(h[n:] @ W_fh.T + x @ W_fx.T + b_fx + b_fh)
            fc = fL * c[:n] + fR * c[n:]
        else:
            fc = 0.0
        i, o, u = np.split(iou, 3, axis=1)
        cn = _sig(i) * np.tanh(u) + fc
        hn = _sig(o) * np.tanh(cn)
        h, c = hn, cn
    return h[0], c[0]


def top_of_tree(h3, c3, inputs, W_ioux, b_ioux, W_iouh, b_iouh,
                W_fx, b_fx, W_fh, b_fh):
    """numpy levels 2..0. h3/c3: [8, 256] states of nodes 7..14."""
    f32 = np.float32
    h = np.zeros((15, D), dtype=f32)
    c = np.zeros((15, D), dtype=f32)
    h[7:15] = h3
    c[7:15] = c3
    x = np.asarray(inputs[:7], f32)
    iou_x = x @ np.asarray(W_ioux, f32).T + b_ioux
    fx = x @ np.asarray(W_fx, f32).T + b_fx

    for lvl in (2, 1, 0):
        start, count = 2 ** lvl - 1, 2 ** lvl
        cs = 2 * start + 1
        ch = h[cs:cs + 2 * count].reshape(count, 2, D)
        cc = c[cs:cs + 2 * count].reshape(count, 2, D)
        iou = iou_x[start:start + count] + ch.sum(axis=1) @ W_iouh.T + b_iouh
        f = _sig(np.einsum("nkm,pm->nkp", ch, W_fh) + b_fh
                 + fx[start:start + count][:, None, :])
        fc_sum = (f * cc).sum(axis=1)
        i, o, u = np.split(iou, 3, axis=1)
        c_new = _sig(i) * np.tanh(u) + fc_sum
        h_new = _sig(o) * np.tanh(c_new)
        c[start:start + count] = c_new
        h[start:start + count] = h_new
    return c[0:1].astype(f32), h[0:1].astype(f32)


def run_device(in_maps, trace=False, repeat=1, pool_off=False):
    from concourse.bass_utils import run_bass_kernel_spmd
    nc = build_program(repeat, pool_off)
    return run_bass_kernel_spmd(nc, in_maps, core_ids=list(range(NCORES)),
                                trace=trace)


def kernel(inputs, W_ioux, b_ioux, W_iouh, b_iouh, W_fx, b_fx, W_fh, b_fh):
    args = (inputs, W_ioux, b_ioux, W_iouh, b_iouh, W_fx, b_fx, W_fh, b_fh)
    in_maps = shard_inputs(*args)
    res = run_device(in_maps)
    h3 = np.stack([res.results[k]["out"][0].reshape(D) for k in range(NCORES)])
    c3 = np.stack([res.results[k]["out"][1].reshape(D) for k in range(NCORES)])
    return top_of_tree(h3, c3, *args)

